# revision 23
# baseline (speedup 1.0000x reference)
"""DCP dehaze (nn_DCPDehazeGenerator) Trainium2 Bass kernel.

Data-parallel over 8 NeuronCores: 2 images per core. Per image:
  dark channel #1 (f32, exact) -> kth_largest threshold + exact top-k
  tie-breaking -> atmospheric light A -> dark channel #2 (bf16) ->
  guided filter (centered bf16 planes; W-box via DVE sliding scans,
  H-box via banded-matrix matmuls on PE with 1/(31*nh) folded into the
  weights) -> J/T/A outputs.

Self-contained: hardcodes shapes [16,3,512,512] f32.
"""
import numpy as np
import concourse.bacc as bacc
import concourse.mybir as mybir
import concourse.tile as tile
from concourse.bass_utils import run_bass_kernel_spmd

dt = mybir.dt
Alu = mybir.AluOpType
Act = mybir.ActivationFunctionType

B_FULL, CH, H, W = 16, 3, 512, 512
NCORES = 8
IMGS = B_FULL // NCORES          # 2 images per core
P, C4 = 128, 4                   # plane layout [128, 4, 512]; h = 128*c + p
K = int(H * W * 0.001)           # 262
EPS, OMEGA = 1e-3, 0.95
PW = 528                         # W-min padded width (7 + 512 + 9)
SB = 560                         # scan pad buffer width (33 zeros + 512 + 15)
SCN = 529                        # scan length; box[w] = scan[w + 17]
SOFF = 17
RBIG = float(1 << 19)            # revIota = RBIG - flat_idx

f32, bf16, f32r = dt.float32, dt.bfloat16, dt.float32r


def _host_consts():
    h = np.arange(H)
    nh = (np.minimum(h + 15, H - 1) - np.maximum(h - 15, 0) + 1).astype(np.float32)
    band = (np.abs(h[:, None] - h[None, :]) <= 15).astype(np.float32)
    # B''[k, m] for out chunk c reading in chunk c' : weight band/(31*nh(h_out))
    def blk(c_out, c_in):
        sub = band[c_in * 128:(c_in + 1) * 128, c_out * 128:(c_out + 1) * 128].copy()
        sub /= (31.0 * nh[c_out * 128:(c_out + 1) * 128])[None, :]
        return sub.astype(np.float32)          # [k, m] = lhsT layout
    bands = np.stack([
        blk(0, 0),                             # 0: mid for out chunk 0 (edge nh)
        blk(1, 1),                             # 1: mid interior (c=1,2 identical)
        blk(3, 3),                             # 2: mid for out chunk 3
        blk(1, 0),                             # 3: prev (in chunk below out chunk)
        blk(1, 2),                             # 4: next
        np.triu(np.ones((128, 128), np.float32), 1),  # 5: Lstrict [k,m]=1 iff k<m
    ])
    nw = nh
    ratio = np.zeros((P, C4, 30), np.float32)
    ratio[:, :, :15] = (31.0 / nw[:15])[None, None, :]
    ratio[:, :, 15:] = (31.0 / nw[497:])[None, None, :]
    # revIota in [128,4,512] layout: flat = (128*c+p)*512 + w
    p_ = np.arange(P)[:, None, None]
    c_ = np.arange(C4)[None, :, None]
    w_ = np.arange(W)[None, None, :]
    riot = (RBIG - ((128 * c_ + p_) * 512 + w_)).astype(np.float32)
    from ml_dtypes import bfloat16
    bandsb = bands[0:5].astype(bfloat16)
    idf = np.eye(128, dtype=np.float32)
    idb = np.eye(128).astype(bfloat16)
    return bands, bandsb, ratio, riot, idf, idb


def _dram_plane(dram_ap):
    """[512,512] dram AP -> [128,4,512] view, h = 128*c + p."""
    return dram_ap.rearrange("(c p) w -> p c w", c=C4, p=P)


def build_nc(n_imgs=IMGS, stage=3):
    nc = bacc.Bacc()
    x_in = nc.declare_dram_parameter("x", [n_imgs, CH, H, W], f32, isOutput=False)
    bands_in = nc.declare_dram_parameter("bands", [6, 128, 128], f32, isOutput=False)
    bandsb_in = nc.declare_dram_parameter("bandsb", [5, 128, 128], bf16, isOutput=False)
    idf_in = nc.declare_dram_parameter("idf", [128, 128], f32, isOutput=False)
    idb_in = nc.declare_dram_parameter("idb", [128, 128], bf16, isOutput=False)
    ratio_in = nc.declare_dram_parameter("ratio", [P, C4, 30], f32, isOutput=False)
    riot_in = nc.declare_dram_parameter("riot", [P, C4, W], f32, isOutput=False)
    J_out = nc.declare_dram_parameter("J", [n_imgs, CH, H, W], f32, isOutput=True)
    T_out = nc.declare_dram_parameter("T", [n_imgs, 1, H, W], f32, isOutput=True)
    A_out = nc.declare_dram_parameter("A", [n_imgs, CH, H, W], f32, isOutput=True)

    from contextlib import ExitStack
    ctx = ExitStack()
    ctx.enter_context(nc.allow_low_precision(reason="centered bf16 guided filter, validated numerically"))
    with tile.TileContext(nc) as tc:
        with (
            tc.tile_pool(name="const", bufs=1) as constp,
            tc.tile_pool(name="x", bufs=1) as xp,
            tc.tile_pool(name="pads", bufs=1) as padp,
            tc.tile_pool(name="wtmp", bufs=2) as wtmpp,
            tc.tile_pool(name="dc", bufs=1) as dcp,
            tc.tile_pool(name="bigf", bufs=2) as bigfp,
            tc.tile_pool(name="mask", bufs=1) as maskp,
            tc.tile_pool(name="mean", bufs=4) as meanp,
            tc.tile_pool(name="chain", bufs=4) as chainp,
            tc.tile_pool(name="zt", bufs=1) as ztp,
            tc.tile_pool(name="scano", bufs=2) as scanop,
            tc.tile_pool(name="scr", bufs=2) as scrp,
            tc.tile_pool(name="small", bufs=2) as smallp,
            tc.tile_pool(name="mm", bufs=8, space="PSUM") as mmp,
        ):
            # ---------------- constants ----------------
            bands = constp.tile([128, 6, 128], f32, tag="bands")
            for i in range(6):
                nc.sync.dma_start(out=bands[:, i, :], in_=bands_in[i])
            bandsb = constp.tile([128, 5, 128], bf16, tag="bandsb")
            for i in range(5):
                nc.sync.dma_start(out=bandsb[:, i, :], in_=bandsb_in[i])
            idf = constp.tile([128, 128], f32, tag="idf")
            nc.sync.dma_start(out=idf[:], in_=idf_in[:])
            idb = constp.tile([128, 128], bf16, tag="idb")
            nc.sync.dma_start(out=idb[:], in_=idb_in[:])
            ratio = constp.tile([P, C4, 30], f32, tag="ratio")
            nc.sync.dma_start(out=ratio[:], in_=ratio_in[:])
            riot = constp.tile([P, C4, W], f32, tag="riot")
            nc.sync.dma_start(out=riot[:], in_=riot_in[:])

            # persistent padded buffers (pads memset once; interiors per image)
            pw1 = padp.tile([P, C4, PW], f32, tag="pw1")        # W-min pad (1.0)
            pw2 = padp.tile([P, C4, PW], bf16, tag="pw2")
            for t in (pw1, pw2):
                nc.vector.memset(t[:, :, 0:7], 1.0)
                nc.vector.memset(t[:, :, 519:PW], 1.0)
            S = padp.tile([P, C4, 513], f32, tag="S")           # tie prefix
            nc.vector.memset(S[:, :, 0:1], 0.0)
            CPb = padp.tile([P, 5], f32, tag="CPb")
            nc.vector.memset(CPb[:, 0:1], 0.0)

            def interior(t):
                return t[:, :, 33:545]



            def wmin(dst, src_pad, eng0, eng1):
                """15-wide W-min: src_pad [P,C4,PW] -> dst [P,C4,512] views."""
                for h0 in (0, 2):
                    sp = src_pad[:, h0:h0 + 2, :]
                    a = wtmpp.tile([P, 2, PW], src_pad.tensor.dtype, tag="wt",
                                   name=f"wma{h0}")
                    b = wtmpp.tile([P, 2, PW], src_pad.tensor.dtype, tag="wt",
                                   name=f"wmb{h0}")
                    eng0.tensor_tensor(out=a[:, :, 0:527], in0=sp[:, :, 0:527],
                                       in1=sp[:, :, 1:528], op=Alu.min)
                    eng1.tensor_tensor(out=b[:, :, 0:525], in0=a[:, :, 0:525],
                                       in1=a[:, :, 2:527], op=Alu.min)
                    eng0.tensor_tensor(out=a[:, :, 0:521], in0=b[:, :, 0:521],
                                       in1=b[:, :, 4:525], op=Alu.min)
                    eng1.tensor_tensor(out=dst[:, h0:h0 + 2, :], in0=a[:, :, 0:512],
                                       in1=a[:, :, 7:519], op=Alu.min)

            def transpose_plane(dst_fn, srcp, ident):
                """PE-transpose [P,4,512] plane; dst_fn(c, psum_ap) evacuates
                each transposed out-chunk c (psum [128,512])."""
                dtp = srcp.tensor.dtype
                for c in range(C4):
                    ps = mmp.tile([P, W], dtp, tag="mm", name=f"tp{c}")
                    for ci in range(C4):
                        nc.tensor.transpose(ps[:, 128 * ci:128 * (ci + 1)],
                                            srcp[:, ci, 128 * c:128 * (c + 1)],
                                            ident[:])
                    dst_fn(c, ps)

            def wbox_scans(scan_out, pad_t):
                """Sliding 31-sum along w. pad_t [P,C4,SB] -> scan_out [P,C4,SCN]."""
                for c in range(C4):
                    nc.vector.tensor_tensor_scan(
                        out=scan_out[:, c, :], data0=pad_t[:, c, 31:31 + SCN],
                        data1=pad_t[:, c, 0:SCN], initial=0.0,
                        op0=Alu.add, op1=Alu.subtract)

            def hbox(mean_dst, scan_t, evac=True):
                """Banded-matmul H-box of scan_t -> bf16 mean planes (+edge fix)."""
                psums = []
                for c in range(C4):
                    ps = mmp.tile([P, W], f32, tag="mm")
                    mid = {0: 0, 1: 1, 2: 1, 3: 2}[c]
                    parts = [(mid, c)]
                    if c > 0:
                        parts.append((3, c - 1))
                    if c < 3:
                        parts.append((4, c + 1))
                    for j, (bi, ci) in enumerate(parts):
                        nc.tensor.matmul(ps[:], bandsb[:, bi, :],
                                         scan_t[:, ci, SOFF:SOFF + W],
                                         start=(j == 0), stop=(j == len(parts) - 1))
                    if evac:
                        nc.scalar.activation(out=mean_dst[:, c, :], in_=ps[:],
                                             func=Act.Copy)
                    else:
                        psums.append(ps)
                if evac:
                    # edge-column ratio fix (w<15, w>=497)
                    nc.vector.tensor_tensor(out=mean_dst[:, :, 0:15],
                                            in0=mean_dst[:, :, 0:15],
                                            in1=ratio[:, :, 0:15], op=Alu.mult)
                    nc.vector.tensor_tensor(out=mean_dst[:, :, 497:512],
                                            in0=mean_dst[:, :, 497:512],
                                            in1=ratio[:, :, 15:30], op=Alu.mult)
                return psums

            # =================== per-image program ===================
            for im in range(n_imgs):
                sIp = padp.tile([P, C4, SB], bf16, tag="spA", name="sIp")
                spp = padp.tile([P, C4, SB], bf16, tag="spB", name="spp")
                sIpp = padp.tile([P, C4, SB], bf16, tag="spC", name="sIpp")
                sII = padp.tile([P, C4, SB], bf16, tag="spD", name="sII")
                if im == 0:
                    for t in (sIp, spp, sIpp, sII):
                        nc.vector.memset(t[:, :, 0:33], 0.0)
                        nc.vector.memset(t[:, :, 545:SB], 0.0)
                xr = xp.tile([P, C4, W], f32, tag="xr")
                xg = xp.tile([P, C4, W], f32, tag="xg")
                xb = xp.tile([P, C4, W], f32, tag="xb")
                nc.sync.dma_start(out=xr[:], in_=_dram_plane(x_in[im, 0]))
                nc.sync.dma_start(out=xg[:], in_=_dram_plane(x_in[im, 1]))
                nc.sync.dma_start(out=xb[:], in_=_dram_plane(x_in[im, 2]))

                # ---- guidance (f32) ----
                guid = bigfp.tile([P, C4, W], f32, tag="bigf")
                g5 = scrp.tile([P, C4, W], f32, tag="scr")
                nc.scalar.activation(out=g5[:], in_=xg[:], func=Act.Copy, scale=0.587)
                nc.vector.scalar_tensor_tensor(out=g5[:], in0=xr[:], scalar=0.2989,
                                               in1=g5[:], op0=Alu.mult, op1=Alu.add)
                nc.vector.scalar_tensor_tensor(out=g5[:], in0=xb[:], scalar=0.114,
                                               in1=g5[:], op0=Alu.mult, op1=Alu.add)
                nc.scalar.activation(out=guid[:], in_=g5[:], func=Act.Copy,
                                     scale=0.5, bias=0.5)

                # ---- dark channel 1 (f32 exact) ----
                t0 = scrp.tile([P, C4, W], f32, tag="scr")
                nc.vector.tensor_tensor(out=t0[:], in0=xr[:], in1=xg[:], op=Alu.min)
                nc.vector.tensor_tensor(out=t0[:], in0=t0[:], in1=xb[:], op=Alu.min)
                nc.scalar.activation(out=pw1[:, :, 7:519], in_=t0[:], func=Act.Copy,
                                     scale=0.5, bias=0.5)
                wmA = padp.tile([P, C4, W], f32, tag="wmA", name="wmA1")
                wmin(wmA[:], pw1, nc.vector, nc.vector)
                transpose_plane(
                    lambda c, ps: nc.scalar.activation(
                        out=pw1[:, c, 7:519], in_=ps[:], func=Act.Copy),
                    wmA, idf)
                wmB = padp.tile([P, C4, W], f32, tag="wmB", name="wmB1")
                wmin(wmB[:], pw1, nc.vector, nc.vector)
                dc = dcp.tile([P, C4, W], f32, tag="dc")
                transpose_plane(
                    lambda c, ps: nc.scalar.activation(
                        out=dc[:, c, :], in_=ps[:], func=Act.Copy),
                    wmB, idf)

                if stage < 1:
                    zP = scrp.tile([P, C4, W], f32, tag="scr", name="zP")
                    nc.vector.memset(zP[:], 0.0)
                    for ch in range(CH):
                        nc.sync.dma_start(out=_dram_plane(A_out[im, ch]), in_=zP[:])
                        nc.sync.dma_start(out=_dram_plane(J_out[im, ch]), in_=zP[:])
                    nc.sync.dma_start(out=_dram_plane(T_out[im, 0]), in_=dc[:])
                    continue
                # ---- top-k threshold + exact selection ----
                kq = smallp.tile([128, 2], f32, tag="kq")
                qm = 1.0 - (float(K) - 1.5) / float(H * W - 1)
                nc.gpsimd.kth_largest(kq[0:1, :], dc[:].rearrange("p c w -> p (c w)"),
                                      n_per_lane=C4 * W, k=300, quantile=qm)
                taub = smallp.tile([128, 1], f32, tag="taub")
                nc.gpsimd.partition_broadcast(taub[:], kq[0:1, 1:2])
                gtm = maskp.tile([P, C4, W], bf16, tag="gtm")
                tiem = maskp.tile([P, C4, W], bf16, tag="tiem", bufs=2)
                gtc = smallp.tile([128, 1], f32, tag="gtc")
                nc.vector.tensor_scalar(out=gtm[:], in0=dc[:], scalar1=taub[:, 0:1],
                                        scalar2=0.0, op0=Alu.is_gt, op1=Alu.add,
                                        accum_out=gtc[:])
                nc.vector.tensor_single_scalar(out=tiem[:], in_=dc[:], scalar=taub[:, 0:1],
                                               op=Alu.is_equal)
                for c in range(C4):
                    nc.vector.tensor_tensor_scan(
                        out=S[:, c, 1:513], data0=tiem[:, c, :], data1=tiem[:, c, :],
                        initial=0.0, op0=Alu.add, op1=Alu.bypass)
                # lex-rank pieces
                Rv = S[:, :, 512]                       # [128,4] per-(p,c) tie counts
                Rvc = smallp.tile([P, C4], f32, tag="Rvc")
                nc.scalar.activation(out=Rvc[:], in_=Rv, func=Act.Copy)
                pp = mmp.tile([P, C4], f32, tag="mm")
                nc.tensor.matmul(pp[:], bands[:, 5, :], Rvc[:],
                                 start=True, stop=True)
                CS = smallp.tile([P, C4], f32, tag="CS")
                from concourse import bass_isa
                nc.gpsimd.partition_all_reduce(CS[:], Rv, channels=128,
                                               reduce_op=bass_isa.ReduceOp.add)
                nc.vector.tensor_tensor_scan(out=CPb[:, 1:5], data0=CS[:],
                                             data1=CS[:], initial=0.0,
                                             op0=Alu.add, op1=Alu.bypass)
                ngt = smallp.tile([128, 1], f32, tag="ngt")
                nc.gpsimd.partition_all_reduce(ngt[:], gtc[:], channels=128,
                                               reduce_op=bass_isa.ReduceOp.add)
                kpp = smallp.tile([128, 1], f32, tag="kpp")
                nc.scalar.activation(out=kpp[:], in_=ngt[:], func=Act.Copy,
                                     scale=-1.0, bias=float(K))
                e0 = smallp.tile([P, C4], f32, tag="e0")
                nc.vector.tensor_tensor(out=e0[:], in0=CPb[:, 0:4], in1=pp[:],
                                        op=Alu.add)
                sel = maskp.tile([P, C4, W], bf16, tag="tiem", bufs=2)
                rks = scrp.tile([P, C4, W], f32, tag="scr", name="rks")
                for c in range(C4):
                    nc.vector.scalar_tensor_tensor(
                        out=rks[:, c, :], in0=S[:, c, 0:512],
                        scalar=e0[:, c:c + 1], op0=Alu.add, op1=Alu.max,
                        in1=S[:, c, 0:512])
                    nc.vector.scalar_tensor_tensor(
                        out=sel[:, c, :], in0=rks[:, c, :], scalar=kpp[:, 0:1],
                        op0=Alu.is_lt, op1=Alu.mult, in1=tiem[:, c, :])
                inS = gtm
                nc.vector.tensor_tensor(out=inS[:], in0=gtm[:], in1=sel[:], op=Alu.add)
                score = bigfp.tile([P, C4, W], f32, tag="bigf")
                mx = smallp.tile([128, 1], f32, tag="mx")
                scrt = scrp.tile([P, C4, W], f32, tag="scr")
                nc.vector.tensor_tensor(out=score[:], in0=inS[:], in1=guid[:],
                                        op=Alu.mult)
                nc.vector.tensor_reduce(out=mx[:], in_=score[:],
                                        axis=mybir.AxisListType.XY, op=Alu.max)
                from concourse import bass_isa as bisa
                mxb = smallp.tile([128, 1], f32, tag="mxb")
                nc.gpsimd.partition_all_reduce(mxb[:], mx[:], channels=128,
                                               reduce_op=bisa.ReduceOp.max)
                brv = smallp.tile([128, 1], f32, tag="brv")
                nc.vector.scalar_tensor_tensor(out=scrt[:], in0=score[:],
                                               scalar=mxb[:, 0:1], op0=Alu.is_equal,
                                               op1=Alu.mult, in1=riot[:],
                                               accum_out=brv[:])
                brva = smallp.tile([128, 1], f32, tag="brva")
                nc.gpsimd.partition_all_reduce(brva[:], brv[:], channels=128,
                                               reduce_op=bisa.ReduceOp.add)
                Astk = smallp.tile([128, 3], f32, tag="Astk")
                for ch, xt in enumerate((xr, xg, xb)):
                    eng = nc.vector
                    eng.scalar_tensor_tensor(out=scrt[:], in0=riot[:],
                                             scalar=brva[:, 0:1], op0=Alu.is_equal,
                                             op1=Alu.mult, in1=xt[:],
                                             accum_out=Astk[:, ch:ch + 1])
                Aall = smallp.tile([128, 3], f32, tag="Aall")   # = map_A per channel
                nc.gpsimd.partition_all_reduce(Aall[:], Astk[:], channels=128,
                                               reduce_op=bisa.ReduceOp.add)
                Asc = smallp.tile([128, 3], f32, tag="Asc")     # A = (map_A+1)/2
                nc.scalar.activation(out=Asc[:], in_=Aall[:], func=Act.Copy,
                                     scale=0.5, bias=0.5)
                invA = smallp.tile([128, 3], f32, tag="invA")
                nc.vector.reciprocal(out=invA[:], in_=Asc[:])
                hinvA = smallp.tile([128, 3], f32, tag="hinvA")
                nc.scalar.activation(out=hinvA[:], in_=invA[:], func=Act.Copy,
                                     scale=0.5)
                bma = smallp.tile([128, 3], f32, tag="bma")     # 0.5 - map_A
                nc.scalar.activation(out=bma[:], in_=Aall[:], func=Act.Copy,
                                     scale=-1.0, bias=0.5)

                if stage < 2:
                    for ch in range(CH):
                        aP0 = scrp.tile([P, C4, W], f32, tag="scr", name="aP0")
                        nc.scalar.activation(out=aP0[:], in_=xr[:], func=Act.Identity,
                                             scale=0.0, bias=Aall[:, ch:ch + 1])
                        nc.sync.dma_start(out=_dram_plane(A_out[im, ch]), in_=aP0[:])
                        nc.sync.dma_start(out=_dram_plane(J_out[im, ch]), in_=aP0[:])
                    nc.sync.dma_start(out=_dram_plane(T_out[im, 0]), in_=dc[:])
                    continue
                # ---- dark channel 2 (bf16) ----
                ch2 = [chainp.tile([P, C4, W], bf16, tag="chain", name=f"ch2_{i}") for i in range(3)]
                for ch, xt in enumerate((xr, xg, xb)):
                    nc.scalar.activation(out=ch2[ch][:], in_=xt[:], func=Act.Identity,
                                         scale=hinvA[:, ch:ch + 1],
                                         bias=hinvA[:, ch:ch + 1])
                m0 = chainp.tile([P, C4, W], bf16, tag="chain")
                nc.vector.tensor_tensor(out=m0[:], in0=ch2[0][:], in1=ch2[1][:],
                                        op=Alu.min)
                nc.vector.tensor_tensor(out=pw2[:, :, 7:519], in0=m0[:],
                                        in1=ch2[2][:], op=Alu.min)
                wmA2 = padp.tile([P, C4, W], bf16, tag="wmA", name="wmA2")
                wmin(wmA2[:], pw2, nc.vector, nc.vector)
                transpose_plane(
                    lambda c, ps: nc.scalar.activation(
                        out=pw2[:, c, 7:519], in_=ps[:], func=Act.Copy),
                    wmA2, idb)
                wmB2 = padp.tile([P, C4, W], bf16, tag="wmB", name="wmB2")
                wmin(wmB2[:], pw2, nc.vector, nc.vector)
                # transpose back, fusing p' = -0.95*minpool into the evac
                spint = interior(spp)
                transpose_plane(
                    lambda c, ps: nc.scalar.activation(
                        out=spint[:, c, :], in_=ps[:], func=Act.Copy, scale=-OMEGA),
                    wmB2, idb)

                # ---- guided filter round 1 (centered bf16) ----
                nc.scalar.activation(out=interior(sIp), in_=guid[:], func=Act.Copy,
                                     scale=1.0, bias=-0.5)
                nc.vector.tensor_tensor(out=interior(sIpp), in0=interior(sIp),
                                        in1=interior(spp), op=Alu.mult)
                nc.scalar.activation(out=interior(sII), in_=interior(sIp),
                                     func=Act.Square)
                so = [scanop.tile([P, C4, SCN], bf16, tag="scano", name=f"so{i}") for i in range(4)]
                for st, pad_t in zip(so, (sIp, spp, sIpp, sII)):
                    wbox_scans(st, pad_t)
                mI = meanp.tile([P, C4, W], bf16, tag="mean")
                mp_ = meanp.tile([P, C4, W], bf16, tag="mean")
                mIp = meanp.tile([P, C4, W], bf16, tag="mean")
                mII = meanp.tile([P, C4, W], bf16, tag="mean")
                for mt, st in zip((mI, mp_, mIp, mII), so):
                    hbox(mt, st)
                # a, b' chain (bf16)
                t1 = chainp.tile([P, C4, W], bf16, tag="chain")
                nc.vector.tensor_tensor(out=t1[:], in0=mI[:], in1=mp_[:], op=Alu.mult)
                cv = chainp.tile([P, C4, W], bf16, tag="chain")
                nc.vector.tensor_tensor(out=cv[:], in0=mIp[:], in1=t1[:],
                                        op=Alu.subtract)
                t2 = chainp.tile([P, C4, W], bf16, tag="chain")
                nc.scalar.activation(out=t2[:], in_=mI[:], func=Act.Square)
                d2 = chainp.tile([P, C4, W], bf16, tag="chain")
                nc.vector.scalar_tensor_tensor(out=d2[:], in0=mII[:], scalar=EPS,
                                               op0=Alu.add, op1=Alu.subtract,
                                               in1=t2[:])
                rp = chainp.tile([P, C4, W], bf16, tag="chain")
                nc.vector.reciprocal(out=rp[:], in_=d2[:])
                spa = padp.tile([P, C4, SB], bf16, tag="spC", name="spa")
                spb = padp.tile([P, C4, SB], bf16, tag="spD", name="spb")
                for t in (spa, spb):
                    nc.vector.memset(t[:, :, 0:33], 0.0)
                    nc.vector.memset(t[:, :, 545:SB], 0.0)
                nc.vector.tensor_tensor(out=interior(spa), in0=cv[:], in1=rp[:],
                                        op=Alu.mult)
                t5 = chainp.tile([P, C4, W], bf16, tag="chain")
                nc.vector.tensor_tensor(out=t5[:], in0=interior(spa), in1=mI[:],
                                        op=Alu.mult)
                nc.vector.tensor_tensor(out=interior(spb), in0=mp_[:], in1=t5[:],
                                        op=Alu.subtract)

                if stage < 3:
                    for ch in range(CH):
                        aP0 = scrp.tile([P, C4, W], f32, tag="scr", name="aP1")
                        nc.scalar.activation(out=aP0[:], in_=xr[:], func=Act.Identity,
                                             scale=0.0, bias=Aall[:, ch:ch + 1])
                        nc.sync.dma_start(out=_dram_plane(A_out[im, ch]), in_=aP0[:])
                        nc.sync.dma_start(out=_dram_plane(J_out[im, ch]), in_=aP0[:])
                    tdbg = bigfp.tile([P, C4, W], f32, tag="bigf", name="tdbg")
                    nc.vector.tensor_tensor(out=tdbg[:], in0=mI[:], in1=mp_[:], op=Alu.add)
                    nc.sync.dma_start(out=_dram_plane(T_out[im, 0]), in_=tdbg[:])
                    continue
                # ---- round 2 + output T, J, A ----
                soa = scanop.tile([P, C4, SCN], bf16, tag="scano")
                sob = scanop.tile([P, C4, SCN], bf16, tag="scano")
                wbox_scans(soa, spa)
                wbox_scans(sob, spb)
                psa = hbox(None, soa, evac=False)
                psb = hbox(None, sob, evac=False)
                z2 = ztp.tile([P, C4, W], f32, tag="zt")
                for c in range(C4):
                    z1c = ztp.tile([P, W], bf16, tag="z1")
                    nc.vector.tensor_tensor(out=z1c[:], in0=psa[c][:],
                                            in1=interior(sIp)[:, c, :], op=Alu.mult)
                    nc.vector.tensor_tensor(out=z2[:, c, :], in0=z1c[:],
                                            in1=psb[c][:], op=Alu.add)
                nc.vector.tensor_tensor(out=z2[:, :, 0:15], in0=z2[:, :, 0:15],
                                        in1=ratio[:, :, 0:15], op=Alu.mult)
                nc.vector.tensor_tensor(out=z2[:, :, 497:512], in0=z2[:, :, 497:512],
                                        in1=ratio[:, :, 15:30], op=Alu.mult)
                Tt = bigfp.tile([P, C4, W], f32, tag="bigf")
                nc.scalar.activation(out=Tt[:], in_=z2[:], func=Act.Identity,
                                     scale=1.0, bias=1.0)
                nc.sync.dma_start(out=_dram_plane(T_out[im, 0]), in_=Tt[:])
                rT = bigfp.tile([P, C4, W], f32, tag="bigf")
                nc.vector.reciprocal(out=rT[:], in_=Tt[:])
                for ch, xt in enumerate((xr, xg, xb)):
                    sA = scrp.tile([P, C4, W], f32, tag="scr")
                    nc.scalar.activation(out=sA[:], in_=xt[:], func=Act.Identity,
                                         scale=0.5, bias=bma[:, ch:ch + 1])
                    mA = scrp.tile([P, C4, W], f32, tag="scr")
                    nc.vector.tensor_tensor(out=mA[:], in0=sA[:], in1=rT[:],
                                            op=Alu.mult)
                    jA = scrp.tile([P, C4, W], f32, tag="scr")
                    nc.scalar.activation(out=jA[:], in_=mA[:], func=Act.Identity,
                                         scale=1.0, bias=Aall[:, ch:ch + 1])
                    nc.sync.dma_start(out=_dram_plane(J_out[im, ch]), in_=jA[:])
                    aP = scrp.tile([P, C4, W], f32, tag="scr")
                    nc.scalar.activation(out=aP[:], in_=xr[:], func=Act.Identity,
                                         scale=0.0, bias=Aall[:, ch:ch + 1])
                    nc.sync.dma_start(out=_dram_plane(A_out[im, ch]), in_=aP[:])
    nc.finalize()
    return nc


_NC_CACHE = {}
_LAST_EXEC_NS = None


def _get_nc():
    if "nc" not in _NC_CACHE:
        _NC_CACHE["nc"] = build_nc()
    return _NC_CACHE["nc"]


def kernel(x):
    x = np.asarray(x, np.float32)
    assert x.shape == (B_FULL, CH, H, W)
    bands, bandsb, ratio, riot, idf, idb = _host_consts()
    nc = _get_nc()
    in_maps = []
    for c in range(NCORES):
        in_maps.append({"x": np.ascontiguousarray(x[c * IMGS:(c + 1) * IMGS]),
                        "bands": bands, "bandsb": bandsb, "ratio": ratio, "riot": riot, "idf": idf, "idb": idb})
    import os
    trace = bool(os.environ.get("KERNEL_TRACE"))
    res = run_bass_kernel_spmd(nc, in_maps, list(range(NCORES)), trace=trace)
    global _LAST_EXEC_NS
    _LAST_EXEC_NS = res.exec_time_ns
    J = np.concatenate([res.results[c]["J"] for c in range(NCORES)], 0)
    T = np.concatenate([res.results[c]["T"] for c in range(NCORES)], 0)
    A = np.concatenate([res.results[c]["A"] for c in range(NCORES)], 0)
    return J, T, A


# revision 24
# speedup vs baseline: 1.2308x; 1.2308x over previous
"""DCP dehaze (nn_DCPDehazeGenerator) Trainium2 Bass kernel.

Data-parallel over 8 NeuronCores: 2 images per core. Per image:
  dark channel #1 (f32, exact) -> kth_largest threshold + exact top-k
  tie-breaking -> atmospheric light A -> dark channel #2 (bf16) ->
  guided filter (centered bf16 planes; W-box via DVE sliding scans,
  H-box via banded-matrix matmuls on PE with 1/(31*nh) folded into the
  weights) -> J/T/A outputs.

Self-contained: hardcodes shapes [16,3,512,512] f32.
"""
import numpy as np
import concourse.bacc as bacc
import concourse.mybir as mybir
import concourse.tile as tile
from concourse.bass_utils import run_bass_kernel_spmd

dt = mybir.dt
Alu = mybir.AluOpType
Act = mybir.ActivationFunctionType

B_FULL, CH, H, W = 16, 3, 512, 512
NCORES = 8
IMGS = B_FULL // NCORES          # 2 images per core
P, C4 = 128, 4                   # plane layout [128, 4, 512]; h = 128*c + p
K = int(H * W * 0.001)           # 262
EPS, OMEGA = 1e-3, 0.95
PW = 528                         # W-min padded width (7 + 512 + 9)
SB = 560                         # scan pad buffer width (33 zeros + 512 + 15)
SCN = 529                        # scan length; box[w] = scan[w + 17]
SOFF = 17
RBIG = float(1 << 19)            # revIota = RBIG - flat_idx

f32, bf16, f32r = dt.float32, dt.bfloat16, dt.float32r


def _host_consts():
    h = np.arange(H)
    nh = (np.minimum(h + 15, H - 1) - np.maximum(h - 15, 0) + 1).astype(np.float32)
    band = (np.abs(h[:, None] - h[None, :]) <= 15).astype(np.float32)
    # B''[k, m] for out chunk c reading in chunk c' : weight band/(31*nh(h_out))
    def blk(c_out, c_in):
        sub = band[c_in * 128:(c_in + 1) * 128, c_out * 128:(c_out + 1) * 128].copy()
        sub /= (31.0 * nh[c_out * 128:(c_out + 1) * 128])[None, :]
        return sub.astype(np.float32)          # [k, m] = lhsT layout
    bands = np.stack([
        blk(0, 0),                             # 0: mid for out chunk 0 (edge nh)
        blk(1, 1),                             # 1: mid interior (c=1,2 identical)
        blk(3, 3),                             # 2: mid for out chunk 3
        blk(1, 0),                             # 3: prev (in chunk below out chunk)
        blk(1, 2),                             # 4: next
        np.triu(np.ones((128, 128), np.float32), 1),  # 5: Lstrict [k,m]=1 iff k<m
    ])
    nw = nh
    ratio = np.zeros((P, C4, 30), np.float32)
    ratio[:, :, :15] = (31.0 / nw[:15])[None, None, :]
    ratio[:, :, 15:] = (31.0 / nw[497:])[None, None, :]
    # revIota in [128,4,512] layout: flat = (128*c+p)*512 + w
    p_ = np.arange(P)[:, None, None]
    c_ = np.arange(C4)[None, :, None]
    w_ = np.arange(W)[None, None, :]
    riot = (RBIG - ((128 * c_ + p_) * 512 + w_)).astype(np.float32)
    from ml_dtypes import bfloat16
    bandsb = bands[0:5].astype(bfloat16)
    idf = np.eye(128, dtype=np.float32)
    idb = np.eye(128).astype(bfloat16)
    return bands, bandsb, ratio, riot, idf, idb


def _dram_plane(dram_ap):
    """[512,512] dram AP -> [128,4,512] view, h = 128*c + p."""
    return dram_ap.rearrange("(c p) w -> p c w", c=C4, p=P)


def build_nc(n_imgs=IMGS, stage=3):
    nc = bacc.Bacc()
    x_in = nc.declare_dram_parameter("x", [n_imgs, CH, H, W], f32, isOutput=False)
    bands_in = nc.declare_dram_parameter("bands", [6, 128, 128], f32, isOutput=False)
    bandsb_in = nc.declare_dram_parameter("bandsb", [5, 128, 128], bf16, isOutput=False)
    idf_in = nc.declare_dram_parameter("idf", [128, 128], f32, isOutput=False)
    idb_in = nc.declare_dram_parameter("idb", [128, 128], bf16, isOutput=False)
    ratio_in = nc.declare_dram_parameter("ratio", [P, C4, 30], f32, isOutput=False)
    riot_in = nc.declare_dram_parameter("riot", [P, C4, W], f32, isOutput=False)
    J_out = nc.declare_dram_parameter("J", [n_imgs, CH, H, W], f32, isOutput=True)
    T_out = nc.declare_dram_parameter("T", [n_imgs, 1, H, W], f32, isOutput=True)
    A_out = nc.declare_dram_parameter("A", [n_imgs, CH, H, W], f32, isOutput=True)

    from contextlib import ExitStack
    ctx = ExitStack()
    ctx.enter_context(nc.allow_low_precision(reason="centered bf16 guided filter, validated numerically"))
    with tile.TileContext(nc) as tc:
        with (
            tc.tile_pool(name="const", bufs=1) as constp,
            tc.tile_pool(name="x", bufs=1) as xp,
            tc.tile_pool(name="pads", bufs=1) as padp,
            tc.tile_pool(name="wtmp", bufs=2) as wtmpp,
            tc.tile_pool(name="dc", bufs=1) as dcp,
            tc.tile_pool(name="bigf", bufs=2) as bigfp,
            tc.tile_pool(name="mask", bufs=1) as maskp,
            tc.tile_pool(name="mean", bufs=4) as meanp,
            tc.tile_pool(name="chain", bufs=4) as chainp,
            tc.tile_pool(name="zt", bufs=1) as ztp,
            tc.tile_pool(name="scano", bufs=2) as scanop,
            tc.tile_pool(name="scr", bufs=2) as scrp,
            tc.tile_pool(name="small", bufs=2) as smallp,
            tc.tile_pool(name="mm", bufs=8, space="PSUM") as mmp,
        ):
            # ---------------- constants ----------------
            bands = constp.tile([128, 6, 128], f32, tag="bands")
            for i in range(6):
                nc.sync.dma_start(out=bands[:, i, :], in_=bands_in[i])
            bandsb = constp.tile([128, 5, 128], bf16, tag="bandsb")
            for i in range(5):
                nc.sync.dma_start(out=bandsb[:, i, :], in_=bandsb_in[i])
            idf = constp.tile([128, 128], f32, tag="idf")
            nc.sync.dma_start(out=idf[:], in_=idf_in[:])
            idb = constp.tile([128, 128], bf16, tag="idb")
            nc.sync.dma_start(out=idb[:], in_=idb_in[:])
            ratio = constp.tile([P, C4, 30], f32, tag="ratio")
            nc.sync.dma_start(out=ratio[:], in_=ratio_in[:])
            riot = constp.tile([P, C4, W], f32, tag="riot")
            nc.sync.dma_start(out=riot[:], in_=riot_in[:])

            # persistent padded buffers (pads memset once; interiors per image)
            pw1 = padp.tile([P, C4, PW], f32, tag="pw1")        # W-min pad (1.0)
            pw2 = padp.tile([P, C4, PW], bf16, tag="pw2")
            for t in (pw1, pw2):
                nc.vector.memset(t[:, :, 0:7], 1.0)
                nc.vector.memset(t[:, :, 519:PW], 1.0)
            S = padp.tile([P, C4, 513], f32, tag="S")           # tie prefix
            nc.vector.memset(S[:, :, 0:1], 0.0)
            CPb = padp.tile([P, 5], f32, tag="CPb")
            nc.vector.memset(CPb[:, 0:1], 0.0)

            def interior(t):
                return t[:, :, 33:545]



            def wmin(dst, src_pad, eng0, eng1):
                """15-wide W-min: src_pad [P,C4,PW] -> dst [P,C4,512] views."""
                for h0 in (0, 2):
                    sp = src_pad[:, h0:h0 + 2, :]
                    a = wtmpp.tile([P, 2, PW], src_pad.tensor.dtype, tag="wt",
                                   name=f"wma{h0}")
                    b = wtmpp.tile([P, 2, PW], src_pad.tensor.dtype, tag="wt",
                                   name=f"wmb{h0}")
                    eng0.tensor_tensor(out=a[:, :, 0:527], in0=sp[:, :, 0:527],
                                       in1=sp[:, :, 1:528], op=Alu.min)
                    eng1.tensor_tensor(out=b[:, :, 0:525], in0=a[:, :, 0:525],
                                       in1=a[:, :, 2:527], op=Alu.min)
                    eng0.tensor_tensor(out=a[:, :, 0:521], in0=b[:, :, 0:521],
                                       in1=b[:, :, 4:525], op=Alu.min)
                    eng1.tensor_tensor(out=dst[:, h0:h0 + 2, :], in0=a[:, :, 0:512],
                                       in1=a[:, :, 7:519], op=Alu.min)

            def transpose_plane(dst_fn, srcp, ident):
                """PE-transpose [P,4,512] plane; dst_fn(c, psum_ap) evacuates
                each transposed out-chunk c (psum [128,512])."""
                dtp = srcp.tensor.dtype
                for c in range(C4):
                    ps = mmp.tile([P, W], dtp, tag="mm", name=f"tp{c}")
                    for ci in range(C4):
                        nc.tensor.transpose(ps[:, 128 * ci:128 * (ci + 1)],
                                            srcp[:, ci, 128 * c:128 * (c + 1)],
                                            ident[:])
                    dst_fn(c, ps)

            def wbox_scans(scan_out, pad_t):
                """Sliding 31-sum along w. pad_t [P,C4,SB] -> scan_out [P,C4,SCN]."""
                for c in range(C4):
                    nc.vector.tensor_tensor_scan(
                        out=scan_out[:, c, :], data0=pad_t[:, c, 31:31 + SCN],
                        data1=pad_t[:, c, 0:SCN], initial=0.0,
                        op0=Alu.add, op1=Alu.subtract)

            def hbox(mean_dst, scan_t, evac=True):
                """Banded-matmul H-box of scan_t -> bf16 mean planes (+edge fix)."""
                psums = []
                for c in range(C4):
                    ps = mmp.tile([P, W], f32, tag="mm")
                    mid = {0: 0, 1: 1, 2: 1, 3: 2}[c]
                    parts = [(mid, c)]
                    if c > 0:
                        parts.append((3, c - 1))
                    if c < 3:
                        parts.append((4, c + 1))
                    for j, (bi, ci) in enumerate(parts):
                        nc.tensor.matmul(ps[:], bandsb[:, bi, :],
                                         scan_t[:, ci, SOFF:SOFF + W],
                                         start=(j == 0), stop=(j == len(parts) - 1))
                    if evac:
                        nc.scalar.activation(out=mean_dst[:, c, :], in_=ps[:],
                                             func=Act.Copy)
                    else:
                        psums.append(ps)
                if evac:
                    # edge-column ratio fix (w<15, w>=497)
                    nc.vector.tensor_tensor(out=mean_dst[:, :, 0:15],
                                            in0=mean_dst[:, :, 0:15],
                                            in1=ratio[:, :, 0:15], op=Alu.mult)
                    nc.vector.tensor_tensor(out=mean_dst[:, :, 497:512],
                                            in0=mean_dst[:, :, 497:512],
                                            in1=ratio[:, :, 15:30], op=Alu.mult)
                return psums

            # =================== per-image program ===================
            for im in range(n_imgs):
                sIp = padp.tile([P, C4, SB], bf16, tag="spA", name="sIp")
                spp = padp.tile([P, C4, SB], bf16, tag="spB", name="spp")
                sIpp = padp.tile([P, C4, SB], bf16, tag="spC", name="sIpp")
                sII = padp.tile([P, C4, SB], bf16, tag="spD", name="sII")
                if im == 0:
                    for t in (sIp, spp, sIpp, sII):
                        nc.vector.memset(t[:, :, 0:33], 0.0)
                        nc.vector.memset(t[:, :, 545:SB], 0.0)
                xr = xp.tile([P, C4, W], f32, tag="xr")
                xg = xp.tile([P, C4, W], f32, tag="xg")
                xb = xp.tile([P, C4, W], f32, tag="xb")
                nc.sync.dma_start(out=xr[:], in_=_dram_plane(x_in[im, 0]))
                nc.sync.dma_start(out=xg[:], in_=_dram_plane(x_in[im, 1]))
                nc.sync.dma_start(out=xb[:], in_=_dram_plane(x_in[im, 2]))

                # ---- guidance (f32) ----
                guid = bigfp.tile([P, C4, W], f32, tag="bigf")
                g5 = scrp.tile([P, C4, W], f32, tag="scr")
                nc.scalar.activation(out=g5[:], in_=xg[:], func=Act.Copy, scale=0.587)
                nc.vector.scalar_tensor_tensor(out=g5[:], in0=xr[:], scalar=0.2989,
                                               in1=g5[:], op0=Alu.mult, op1=Alu.add)
                nc.vector.scalar_tensor_tensor(out=g5[:], in0=xb[:], scalar=0.114,
                                               in1=g5[:], op0=Alu.mult, op1=Alu.add)
                nc.scalar.activation(out=guid[:], in_=g5[:], func=Act.Copy,
                                     scale=0.5, bias=0.5)

                # ---- dark channel 1 (f32 exact) ----
                t0 = scrp.tile([P, C4, W], f32, tag="scr")
                nc.vector.tensor_tensor(out=t0[:], in0=xr[:], in1=xg[:], op=Alu.min)
                nc.vector.tensor_tensor(out=t0[:], in0=t0[:], in1=xb[:], op=Alu.min)
                nc.scalar.activation(out=pw1[:, :, 7:519], in_=t0[:], func=Act.Copy,
                                     scale=0.5, bias=0.5)
                wmA = padp.tile([P, C4, W], f32, tag="wmA", name="wmA1")
                wmin(wmA[:], pw1, nc.vector, nc.vector)
                transpose_plane(
                    lambda c, ps: nc.scalar.activation(
                        out=pw1[:, c, 7:519], in_=ps[:], func=Act.Copy),
                    wmA, idf)
                wmB = padp.tile([P, C4, W], f32, tag="wmB", name="wmB1")
                wmin(wmB[:], pw1, nc.vector, nc.vector)
                dc = dcp.tile([P, C4, W], f32, tag="dc")
                transpose_plane(
                    lambda c, ps: nc.scalar.activation(
                        out=dc[:, c, :], in_=ps[:], func=Act.Copy),
                    wmB, idf)

                if stage < 1:
                    zP = scrp.tile([P, C4, W], f32, tag="scr", name="zP")
                    nc.vector.memset(zP[:], 0.0)
                    for ch in range(CH):
                        nc.sync.dma_start(out=_dram_plane(A_out[im, ch]), in_=zP[:])
                        nc.sync.dma_start(out=_dram_plane(J_out[im, ch]), in_=zP[:])
                    nc.sync.dma_start(out=_dram_plane(T_out[im, 0]), in_=dc[:])
                    continue
                # ---- top-k threshold + exact selection ----
                kq = smallp.tile([128, 2], f32, tag="kq")
                qm = 1.0 - (float(K) - 1.5) / float(H * W - 1)
                nc.gpsimd.kth_largest(kq[0:1, :], dc[:].rearrange("p c w -> p (c w)"),
                                      n_per_lane=C4 * W, k=300, quantile=qm)
                taub = smallp.tile([128, 1], f32, tag="taub")
                nc.gpsimd.partition_broadcast(taub[:], kq[0:1, 1:2])
                gtm = maskp.tile([P, C4, W], bf16, tag="gtm")
                tiem = maskp.tile([P, C4, W], bf16, tag="tiem", bufs=2)
                gtc = smallp.tile([128, 1], f32, tag="gtc")
                nc.vector.tensor_scalar(out=gtm[:], in0=dc[:], scalar1=taub[:, 0:1],
                                        scalar2=0.0, op0=Alu.is_gt, op1=Alu.add,
                                        accum_out=gtc[:])
                nc.vector.tensor_single_scalar(out=tiem[:], in_=dc[:], scalar=taub[:, 0:1],
                                               op=Alu.is_equal)
                for c in range(C4):
                    nc.vector.tensor_tensor_scan(
                        out=S[:, c, 1:513], data0=tiem[:, c, :], data1=tiem[:, c, :],
                        initial=0.0, op0=Alu.add, op1=Alu.bypass)
                # lex-rank pieces
                Rv = S[:, :, 512]                       # [128,4] per-(p,c) tie counts
                Rvc = smallp.tile([P, C4], f32, tag="Rvc")
                nc.scalar.activation(out=Rvc[:], in_=Rv, func=Act.Copy)
                pp = mmp.tile([P, C4], f32, tag="mm")
                nc.tensor.matmul(pp[:], bands[:, 5, :], Rvc[:],
                                 start=True, stop=True)
                CS = smallp.tile([P, C4], f32, tag="CS")
                from concourse import bass_isa
                nc.gpsimd.partition_all_reduce(CS[:], Rv, channels=128,
                                               reduce_op=bass_isa.ReduceOp.add)
                nc.vector.tensor_tensor_scan(out=CPb[:, 1:5], data0=CS[:],
                                             data1=CS[:], initial=0.0,
                                             op0=Alu.add, op1=Alu.bypass)
                ngt = smallp.tile([128, 1], f32, tag="ngt")
                nc.gpsimd.partition_all_reduce(ngt[:], gtc[:], channels=128,
                                               reduce_op=bass_isa.ReduceOp.add)
                kpp = smallp.tile([128, 1], f32, tag="kpp")
                nc.scalar.activation(out=kpp[:], in_=ngt[:], func=Act.Copy,
                                     scale=-1.0, bias=float(K))
                e0 = smallp.tile([P, C4], f32, tag="e0")
                nc.vector.tensor_tensor(out=e0[:], in0=CPb[:, 0:4], in1=pp[:],
                                        op=Alu.add)
                sel = maskp.tile([P, C4, W], bf16, tag="tiem", bufs=2)
                rks = scrp.tile([P, C4, W], f32, tag="scr", name="rks")
                for c in range(C4):
                    nc.vector.scalar_tensor_tensor(
                        out=rks[:, c, :], in0=S[:, c, 0:512],
                        scalar=e0[:, c:c + 1], op0=Alu.add, op1=Alu.max,
                        in1=S[:, c, 0:512])
                    nc.vector.scalar_tensor_tensor(
                        out=sel[:, c, :], in0=rks[:, c, :], scalar=kpp[:, 0:1],
                        op0=Alu.is_lt, op1=Alu.mult, in1=tiem[:, c, :])
                inS = gtm
                nc.vector.tensor_tensor(out=inS[:], in0=gtm[:], in1=sel[:], op=Alu.add)
                score = bigfp.tile([P, C4, W], f32, tag="bigf")
                mx = smallp.tile([128, 1], f32, tag="mx")
                scrt = scrp.tile([P, C4, W], f32, tag="scr")
                nc.vector.tensor_tensor(out=score[:], in0=inS[:], in1=guid[:],
                                        op=Alu.mult)
                nc.vector.tensor_reduce(out=mx[:], in_=score[:],
                                        axis=mybir.AxisListType.XY, op=Alu.max)
                from concourse import bass_isa as bisa
                mxb = smallp.tile([128, 1], f32, tag="mxb")
                nc.gpsimd.partition_all_reduce(mxb[:], mx[:], channels=128,
                                               reduce_op=bisa.ReduceOp.max)
                brv = smallp.tile([128, 1], f32, tag="brv")
                nc.vector.scalar_tensor_tensor(out=scrt[:], in0=score[:],
                                               scalar=mxb[:, 0:1], op0=Alu.is_equal,
                                               op1=Alu.mult, in1=riot[:],
                                               accum_out=brv[:])
                brva = smallp.tile([128, 1], f32, tag="brva")
                nc.gpsimd.partition_all_reduce(brva[:], brv[:], channels=128,
                                               reduce_op=bisa.ReduceOp.add)
                Astk = smallp.tile([128, 3], f32, tag="Astk")
                for ch, xt in enumerate((xr, xg, xb)):
                    eng = nc.vector
                    eng.scalar_tensor_tensor(out=scrt[:], in0=riot[:],
                                             scalar=brva[:, 0:1], op0=Alu.is_equal,
                                             op1=Alu.mult, in1=xt[:],
                                             accum_out=Astk[:, ch:ch + 1])
                Aall = smallp.tile([128, 3], f32, tag="Aall")   # = map_A per channel
                nc.gpsimd.partition_all_reduce(Aall[:], Astk[:], channels=128,
                                               reduce_op=bisa.ReduceOp.add)
                Asc = smallp.tile([128, 3], f32, tag="Asc")     # A = (map_A+1)/2
                nc.scalar.activation(out=Asc[:], in_=Aall[:], func=Act.Copy,
                                     scale=0.5, bias=0.5)
                invA = smallp.tile([128, 3], f32, tag="invA")
                nc.vector.reciprocal(out=invA[:], in_=Asc[:])
                hinvA = smallp.tile([128, 3], f32, tag="hinvA")
                nc.scalar.activation(out=hinvA[:], in_=invA[:], func=Act.Copy,
                                     scale=0.5)
                bma = smallp.tile([128, 3], f32, tag="bma")     # 0.5 - map_A
                nc.scalar.activation(out=bma[:], in_=Aall[:], func=Act.Copy,
                                     scale=-1.0, bias=0.5)

                if stage < 2:
                    for ch in range(CH):
                        aP0 = scrp.tile([P, C4, W], f32, tag="scr", name="aP0")
                        nc.scalar.activation(out=aP0[:], in_=xr[:], func=Act.Identity,
                                             scale=0.0, bias=Aall[:, ch:ch + 1])
                        nc.sync.dma_start(out=_dram_plane(A_out[im, ch]), in_=aP0[:])
                        nc.sync.dma_start(out=_dram_plane(J_out[im, ch]), in_=aP0[:])
                    nc.sync.dma_start(out=_dram_plane(T_out[im, 0]), in_=dc[:])
                    continue
                # ---- dark channel 2 (bf16) ----
                ch2 = [chainp.tile([P, C4, W], bf16, tag="chain", name=f"ch2_{i}") for i in range(3)]
                for ch, xt in enumerate((xr, xg, xb)):
                    nc.scalar.activation(out=ch2[ch][:], in_=xt[:], func=Act.Identity,
                                         scale=hinvA[:, ch:ch + 1],
                                         bias=hinvA[:, ch:ch + 1])
                m0 = chainp.tile([P, C4, W], bf16, tag="chain")
                nc.vector.tensor_tensor(out=m0[:], in0=ch2[0][:], in1=ch2[1][:],
                                        op=Alu.min)
                nc.vector.tensor_tensor(out=pw2[:, :, 7:519], in0=m0[:],
                                        in1=ch2[2][:], op=Alu.min)
                wmA2 = padp.tile([P, C4, W], bf16, tag="wmA", name="wmA2")
                wmin(wmA2[:], pw2, nc.vector, nc.vector)
                transpose_plane(
                    lambda c, ps: nc.scalar.activation(
                        out=pw2[:, c, 7:519], in_=ps[:], func=Act.Copy),
                    wmA2, idb)
                wmB2 = padp.tile([P, C4, W], bf16, tag="wmB", name="wmB2")
                wmin(wmB2[:], pw2, nc.vector, nc.vector)
                # transpose back, fusing p' = -0.95*minpool into the evac
                spint = interior(spp)
                transpose_plane(
                    lambda c, ps: nc.scalar.activation(
                        out=spint[:, c, :], in_=ps[:], func=Act.Copy, scale=-OMEGA),
                    wmB2, idb)

                # ---- guided filter round 1 (centered bf16) ----
                nc.scalar.activation(out=interior(sIp), in_=guid[:], func=Act.Copy,
                                     scale=1.0, bias=-0.5)
                nc.vector.tensor_tensor(out=interior(sIpp), in0=interior(sIp),
                                        in1=interior(spp), op=Alu.mult)
                nc.scalar.activation(out=interior(sII), in_=interior(sIp),
                                     func=Act.Square)
                so = [scanop.tile([P, C4, SCN], bf16, tag="scano", name=f"so{i}") for i in range(4)]
                for st, pad_t in zip(so, (sIp, spp, sIpp, sII)):
                    wbox_scans(st, pad_t)
                mI = meanp.tile([P, C4, W], bf16, tag="mean")
                mp_ = meanp.tile([P, C4, W], bf16, tag="mean")
                mIp = meanp.tile([P, C4, W], bf16, tag="mean")
                mII = meanp.tile([P, C4, W], bf16, tag="mean")
                for mt, st in zip((mI, mp_, mIp, mII), so):
                    hbox(mt, st)
                # a, b' chain (bf16)
                t1 = chainp.tile([P, C4, W], bf16, tag="chain")
                nc.vector.tensor_tensor(out=t1[:], in0=mI[:], in1=mp_[:], op=Alu.mult)
                cv = chainp.tile([P, C4, W], bf16, tag="chain")
                nc.vector.tensor_tensor(out=cv[:], in0=mIp[:], in1=t1[:],
                                        op=Alu.subtract)
                t2 = chainp.tile([P, C4, W], bf16, tag="chain")
                nc.scalar.activation(out=t2[:], in_=mI[:], func=Act.Square)
                d2 = chainp.tile([P, C4, W], bf16, tag="chain")
                nc.vector.scalar_tensor_tensor(out=d2[:], in0=mII[:], scalar=EPS,
                                               op0=Alu.add, op1=Alu.subtract,
                                               in1=t2[:])
                rp = chainp.tile([P, C4, W], bf16, tag="chain")
                nc.vector.reciprocal(out=rp[:], in_=d2[:])
                spa = padp.tile([P, C4, SB], bf16, tag="spC", name="spa")
                spb = padp.tile([P, C4, SB], bf16, tag="spD", name="spb")
                for t in (spa, spb):
                    nc.vector.memset(t[:, :, 0:33], 0.0)
                    nc.vector.memset(t[:, :, 545:SB], 0.0)
                nc.vector.tensor_tensor(out=interior(spa), in0=cv[:], in1=rp[:],
                                        op=Alu.mult)
                t5 = chainp.tile([P, C4, W], bf16, tag="chain")
                nc.vector.tensor_tensor(out=t5[:], in0=interior(spa), in1=mI[:],
                                        op=Alu.mult)
                nc.vector.tensor_tensor(out=interior(spb), in0=mp_[:], in1=t5[:],
                                        op=Alu.subtract)

                if stage < 3:
                    for ch in range(CH):
                        aP0 = scrp.tile([P, C4, W], f32, tag="scr", name="aP1")
                        nc.scalar.activation(out=aP0[:], in_=xr[:], func=Act.Identity,
                                             scale=0.0, bias=Aall[:, ch:ch + 1])
                        nc.sync.dma_start(out=_dram_plane(A_out[im, ch]), in_=aP0[:])
                        nc.sync.dma_start(out=_dram_plane(J_out[im, ch]), in_=aP0[:])
                    tdbg = bigfp.tile([P, C4, W], f32, tag="bigf", name="tdbg")
                    nc.vector.tensor_tensor(out=tdbg[:], in0=mI[:], in1=mp_[:], op=Alu.add)
                    nc.sync.dma_start(out=_dram_plane(T_out[im, 0]), in_=tdbg[:])
                    continue
                # ---- round 2 + output T, J, A ----
                soa = scanop.tile([P, C4, SCN], bf16, tag="scano")
                sob = scanop.tile([P, C4, SCN], bf16, tag="scano")
                wbox_scans(soa, spa)
                wbox_scans(sob, spb)
                psa = hbox(None, soa, evac=False)
                psb = hbox(None, sob, evac=False)
                z2 = ztp.tile([P, C4, W], f32, tag="zt")
                for c in range(C4):
                    z1c = ztp.tile([P, W], bf16, tag="z1")
                    nc.vector.tensor_tensor(out=z1c[:], in0=psa[c][:],
                                            in1=interior(sIp)[:, c, :], op=Alu.mult)
                    nc.vector.tensor_tensor(out=z2[:, c, :], in0=z1c[:],
                                            in1=psb[c][:], op=Alu.add)
                nc.vector.tensor_tensor(out=z2[:, :, 0:15], in0=z2[:, :, 0:15],
                                        in1=ratio[:, :, 0:15], op=Alu.mult)
                nc.vector.tensor_tensor(out=z2[:, :, 497:512], in0=z2[:, :, 497:512],
                                        in1=ratio[:, :, 15:30], op=Alu.mult)
                Tt = bigfp.tile([P, C4, W], f32, tag="bigf")
                nc.scalar.activation(out=Tt[:], in_=z2[:], func=Act.Identity,
                                     scale=1.0, bias=1.0)
                nc.sync.dma_start(out=_dram_plane(T_out[im, 0]), in_=Tt[:])
                rT = bigfp.tile([P, C4, W], f32, tag="bigf")
                nc.vector.reciprocal(out=rT[:], in_=Tt[:])
                for ch, xt in enumerate((xr, xg, xb)):
                    sA = scrp.tile([P, C4, W], f32, tag="scr")
                    nc.scalar.activation(out=sA[:], in_=xt[:], func=Act.Identity,
                                         scale=0.5, bias=bma[:, ch:ch + 1])
                    mA = scrp.tile([P, C4, W], f32, tag="scr")
                    nc.vector.tensor_tensor(out=mA[:], in0=sA[:], in1=rT[:],
                                            op=Alu.mult)
                    jA = scrp.tile([P, C4, W], f32, tag="scr")
                    nc.scalar.activation(out=jA[:], in_=mA[:], func=Act.Identity,
                                         scale=1.0, bias=Aall[:, ch:ch + 1])
                    nc.sync.dma_start(out=_dram_plane(J_out[im, ch]), in_=jA[:])
                    aP = scrp.tile([P, C4, W], f32, tag="scr")
                    nc.scalar.activation(out=aP[:], in_=xr[:], func=Act.Identity,
                                         scale=0.0, bias=Aall[:, ch:ch + 1])
                    nc.sync.dma_start(out=_dram_plane(A_out[im, ch]), in_=aP[:])
    nc.finalize()
    return nc


_NC_CACHE = {}
_LAST_EXEC_NS = None


def _get_nc():
    if "nc" not in _NC_CACHE:
        _NC_CACHE["nc"] = build_nc()
    return _NC_CACHE["nc"]


def kernel(x):
    x = np.asarray(x, np.float32)
    assert x.shape == (B_FULL, CH, H, W)
    bands, bandsb, ratio, riot, idf, idb = _host_consts()
    nc = _get_nc()
    in_maps = []
    for c in range(NCORES):
        in_maps.append({"x": np.ascontiguousarray(x[c * IMGS:(c + 1) * IMGS]),
                        "bands": bands, "bandsb": bandsb, "ratio": ratio, "riot": riot, "idf": idf, "idb": idb})
    res = run_bass_kernel_spmd(nc, in_maps, list(range(NCORES)))
    global _LAST_EXEC_NS
    _LAST_EXEC_NS = res.exec_time_ns
    J = np.concatenate([res.results[c]["J"] for c in range(NCORES)], 0)
    T = np.concatenate([res.results[c]["T"] for c in range(NCORES)], 0)
    A = np.concatenate([res.results[c]["A"] for c in range(NCORES)], 0)
    return J, T, A
